# revision 59
# baseline (speedup 1.0000x reference)
"""Trainium2 Bass kernel for nn_DeChunkLayer (ragged EMA de-chunk).

Math (per batch row b):
    p[l]   = clip(boundary_prob[b, l, 1], EPS, 1-EPS)
    nb[l]  = cumsum_l(boundary_mask[b])          (>= 1 since l=0 is a boundary)
    h(k)   = (1-pb[k]) h(k-1) + pb[k] x[k]       (EMA over chunk rank k;
                                                  pb = p at the k-th boundary)
    out[l] = h(nb[l]-1)

Compact-scan + hybrid gather/Sel-matmul expansion (v3, 92.3us vs the
144us fused-scan baseline; mem roofline of the fused design ~94us but
this design moves only ~18MB/core):
  Only ~25% of positions are boundaries, so the EMA has only K ~= 2048
  distinct states and its inputs are x[0:K] read SEQUENTIALLY (the k-th
  EMA step uses x row k, not a gathered row).  Host precomputes the tiny
  per-row index arrays (cumsum nb, gather index idx[l] = nb[l]-1,
  compacted pb[k] = p at the k-th boundary, padded with 0 to K_max so
  padded steps are identity h = 1*h + 0*x).  Device then:
    phase A (compact scan over K_max ~= 2560 instead of 8192):
      per 512-k chunk: ONE sequential x DMA (bf16; HWDGE issue is
      ~0.6us/instruction so DMAs are merged), scale+transpose fused as
      one PE matmul per 128-tile against diag(pb) (frees DVE for the
      scans), DVE tensor_tensor_scan per d-tile with the carry read
      directly from the previous chunk's outT[:,511:512] (no copy), PE
      transpose back, ACT downcast-copy into persistent bf16 h_sb, and
      for k < hd_hi a DMA to DRAM scratch hd (scalar-engine DGE - off
      the sync DGE's in-order stream).
    expansion, split across two engines that run concurrently:
      l-chunks < SEL_START: out[l] = hd[idx[l]] via one dma_gather per
      512-l chunk (SWDGE ucode, int16 idx wrapped 16-per-column and
      replicated 8x down partitions; ~4.6us/call = 994ns fixed +
      ~7ns/descriptor - the same descriptor-gen floor as 4 indirect_dma
      calls, but 4x fewer instructions).  hd only holds rows < hd_hi =
      max idx referenced by these chunks (host bound, ~1024): with the
      whole-tensor DRAM dep tracking, gathers then only wait for the
      first hd_hi/512 scan chunks.
      l-chunks >= SEL_START: one-hot Sel matmuls on the otherwise-idle
      PE: psum[a,d] = sum_w Sel_{j,w}^T @ h_sb[ktile u_j+w], where the
      per-l-tile k-window [u_j, u_j+W_j) is computed on host, shared
      across all 4 rows (so one program serves all 8 cores), and baked
      in at (runtime) compile; Sel one-hots are host-built bf16 uploads.
      One-hot bf16 matmuls reproduce h exactly (no accumulation error).
      PSUM->SBUF copies alternate ACT/DVE; out-DMAs alternate the two
      HWDGE engines (sync/scalar).
  gpsimd runs ONLY mlp-library instructions (identity matrix is a host
  upload, not gpsimd memset/affine_select) so its ucode library loads
  once at t~0 instead of stalling the first gather mid-kernel.
  Output is bf16 on device, upconverted to f32 on host (rel err 2.4e-3,
  budget 2e-2).
  Measured (engine-busy at 93us wall): GpSimd 28us (6 gathers),
  TensorMatrix 54us, DVE 42us (scans 24), ACT 31us, DMA queues ~54%.
  Things measured SLOWER: bf16 scan-out + bf16-PSUM transposes (101us),
  per-l-tile sel out-DMAs (102us), SEL_START=7 (100us), selp=3/po=1
  PSUM rebalance (105us), interleaved scan/gather emission (false WAR
  serialization via whole-tensor DRAM dep tracking, 134us).

kernel(**inputs) takes FULL inputs, shards over 8 cores (4 batch rows x 2
D-halves), returns FULL (4, 8192, 1024) f32 output.  The Bass program is
compiled on first call per (K_max, sel windows, hd_hi) signature - all
derived from the boundary masks, so any input with the same signature
reuses the cached NEFF and different data simply recompiles.
"""

import os
import sys

import numpy as np

sys.path.insert(0, "/opt/trn_rl_repo")

B, L, D = 4, 8192, 1024
NCORES = 8
DSH = D // 2          # 512 channels per core
NLT = L // 128        # 64 l-tiles of 128
NLC = L // 512        # 16 l-chunks of 512
NDT = DSH // 128      # 4 d-tiles of 128
EPS = 1e-4

_progs = {}  # (kmax, sel_key) -> compiled Bass program

SEL_START = 6  # l-chunks >= this are expanded via PE one-hot matmuls


def _build_program(kmax, sel_key, hd_hi):
    import concourse.bass as bass
    import concourse.mybir as mybir
    from concourse import bacc
    from concourse.tile import TileContext

    f32 = mybir.dt.float32
    bf16 = mybir.dt.bfloat16
    i32 = mybir.dt.int32
    Op = mybir.AluOpType

    KC = kmax // 512      # k-chunks
    KT = kmax // 128      # k-tiles

    nc = bacc.Bacc("TRN2", target_bir_lowering=False, debug=False,
                   num_devices=NCORES)

    i16 = mybir.dt.int16
    totW = sum(w for _, w in sel_key)
    x = nc.declare_dram_parameter("x", [kmax, DSH], bf16, isOutput=False)
    arow_d = nc.declare_dram_parameter("arow", [1, kmax], f32, isOutput=False)
    pbcm_d = nc.declare_dram_parameter("pbcm", [128, KT], f32, isOutput=False)
    sel_d = nc.declare_dram_parameter("sel", [128, totW * 128], bf16,
                                      isOutput=False)
    ident_d = nc.declare_dram_parameter("ident", [128, 128], f32,
                                        isOutput=False)
    # idx16: [128, NLC*32] int16; per l-chunk slice [:, 32c:32c+32] holds the
    # 512 gather indices wrapped 16-per-column (idx i at partition i%16,
    # col i//16), replicated 8x down the partitions for the 8 gpsimd cores.
    idx16_d = nc.declare_dram_parameter("idx16", [128, NLC * 32], i16,
                                        isOutput=False)
    out = nc.declare_dram_parameter("out", [L, DSH], bf16, isOutput=True)

    with TileContext(nc) as tc:
        with (
            tc.tile_pool(name="const", bufs=1) as cpool,
            tc.tile_pool(name="prep", bufs=1) as ppool,
            tc.tile_pool(name="hdp", bufs=1, space="DRAM") as hdp,
        ):
            # identity uploaded from host: keeps gpsimd free of non-mlp
            # library instructions (its ucode library reload would other-
            # wise stall the first dma_gather ~10us mid-kernel)
            ident = cpool.tile([128, 128], f32, tag="ident")
            nc.sync.dma_start(out=ident[:], in_=ident_d[:])
            identb = cpool.tile([128, 128], bf16, tag="identb")
            nc.vector.tensor_copy(identb[:], ident[:])
            ones1 = cpool.tile([1, 128], f32, tag="ones1")
            nc.vector.memset(ones1[:], 1.0)

            arow = ppool.tile([1, kmax], f32, tag="arow")
            nc.sync.dma_start(out=arow[:], in_=arow_d[:])
            pbcm = ppool.tile([128, KT], f32, tag="pbcm")
            nc.sync.dma_start(out=pbcm[:], in_=pbcm_d[:])
            idx16 = ppool.tile([128, NLC * 32], i16, tag="idx16")
            nc.sync.dma_start(out=idx16[:], in_=idx16_d[:])
            sel_sb = ppool.tile([128, totW * 128], bf16, tag="sel_sb")
            nc.scalar.dma_start(out=sel_sb[:], in_=sel_d[:])
            # persistent k-major h (bf16) — rhs of the Sel expansion matmuls
            h_sb = ppool.tile([128, KT * 512], bf16, tag="h_sb")
            # diag(pb) per k-tile: fuses the bn = pb*x scale into the PE
            # transpose (regular matmul with a diagonal rhs), freeing DVE.
            diag = ppool.tile([128, KT * 128], bf16, tag="diag")
            for t in range(KT):
                nc.vector.tensor_scalar_mul(
                    diag[:][:, 128 * t:128 * (t + 1)], identb[:],
                    pbcm[:][:, t:t + 1])

            # Only the hd prefix the gather chunks reference (host-verified
            # bound on idx over l < 512*SEL_START) goes to DRAM; gathers
            # then wait only on the first hd_hi/512 scan chunks.
            hd = hdp.tile([hd_hi, DSH], bf16, tag="hd", name="hd")

            # chunk-to-chunk scan carries: previous chunk's outT last
            # column is read directly as `initial` (outT pool is 8 deep =
            # 2 chunks, so the slice is alive when the next scan starts)
            prev_outTs = [None] * NDT

            with (
                tc.tile_pool(name="xg", bufs=4) as xgp,
                tc.tile_pool(name="bt", bufs=4, space="PSUM") as btp,
                tc.tile_pool(name="asb", bufs=2) as asbp,
                tc.tile_pool(name="outT", bufs=8) as otp,
                tc.tile_pool(name="po", bufs=2, space="PSUM") as pop,
                tc.tile_pool(name="selp", bufs=2, space="PSUM") as selpp,
                tc.tile_pool(name="gx", bufs=4) as gxp,
                tc.tile_pool(name="gxs", bufs=6) as gxsp,
            ):
                def scan_chunk(c):
                    # broadcast a = (1-pb)[chunk] to 128 partitions
                    apm = pop.tile([128, 512], f32, tag="po",
                                   name=f"apm_{c}")
                    nc.tensor.matmul(
                        out=apm[:], lhsT=ones1[:],
                        rhs=arow[:][0:1, 512 * c:512 * (c + 1)],
                        start=True, stop=True)
                    a_sb = asbp.tile([128, 512], f32, tag="a_sb",
                                     name=f"asb_{c}")
                    nc.scalar.copy(out=a_sb[:], in_=apm[:])

                    # sequential x rows for this chunk (bf16), one DMA
                    # (sequencer issue costs ~0.6us per DMA instruction)
                    xg4 = xgp.tile([128, 4 * DSH], bf16, tag="xg",
                                   name=f"xg_{c}")
                    nc.sync.dma_start(
                        out=xg4[:].rearrange("p (b d) -> p b d", b=4),
                        in_=x[:][512 * c:512 * (c + 1), :].rearrange(
                            "(b p) d -> p b d", p=128))

                    bts = [btp.tile([128, 512], f32, tag="bt",
                                    name=f"bt{t}_{c}")
                           for t in range(NDT)]
                    for kt in range(4):
                        t_col = 4 * c + kt
                        for t in range(NDT):
                            # bts[t][d, k'] = sum_k x[k, d] * diag[k, k']
                            #              = x[k', d] * pb[k']
                            nc.tensor.matmul(
                                out=bts[t][:][:, 128 * kt:128 * (kt + 1)],
                                lhsT=xg4[:][:, DSH * kt + 128 * t:
                                            DSH * kt + 128 * (t + 1)],
                                rhs=diag[:][:, 128 * t_col:128 * (t_col + 1)],
                                start=True, stop=True)

                    outTs = [otp.tile([128, 512], f32, tag=f"outT{t}",
                                      name=f"outT{t}_{c}")
                             for t in range(NDT)]
                    for t in range(NDT):
                        nc.vector.tensor_tensor_scan(
                            out=outTs[t][:], data0=a_sb[:], data1=bts[t][:],
                            initial=(0.0 if c == 0
                                     else prev_outTs[t][:][:, 511:512]),
                            op0=Op.mult, op1=Op.add)
                        prev_outTs[t] = outTs[t]

                    # transpose back to [k, d] rows, downcast to bf16, store
                    # into the persistent h_sb (Sel matmul rhs + hd source)
                    for kt in range(4):
                        po = pop.tile([128, DSH], f32, tag="po",
                                      name=f"po_{c}_{kt}")
                        for t in range(NDT):
                            nc.tensor.transpose(
                                out=po[:][:, 128 * t:128 * (t + 1)],
                                in_=outTs[t][:][:, 128 * kt:128 * (kt + 1)],
                                identity=ident[:])
                        nc.scalar.copy(
                            out=h_sb[:][:, (4 * c + kt) * DSH:
                                        (4 * c + kt + 1) * DSH], in_=po[:])
                    # scalar-engine DGE: keeps hd writes off the sync
                    # sequencer's in-order stream (x loads / out writes)
                    if 512 * c < hd_hi:
                        nc.scalar.dma_start(
                            out=hd[:][512 * c:512 * (c + 1), :].rearrange(
                                "(b a) d -> a b d", a=128),
                            in_=h_sb[:][:, 4 * c * DSH:4 * (c + 1) * DSH]
                            .rearrange("a (b d) -> a b d", b=4))

                def expand_chunk(c2):
                    # h rows needed by l-chunk c2 are < 512*(c2+1) since
                    # idx[l] <= l; slice the gather src so the DRAM dep
                    # tracker only orders against already-written chunks.
                    hi = min(512 * (c2 + 1), hd_hi)
                    g4 = gxp.tile([128, 4 * DSH], bf16, tag="gx",
                                  name=f"gx_{c2}")
                    nc.gpsimd.dma_gather(
                        out_ap=g4[:].rearrange("a (b d) -> a b d", b=4),
                        in_ap=hd[:][0:hi, :],
                        idxs_ap=idx16[:][:, 32 * c2:32 * (c2 + 1)],
                        num_idxs=512, num_idxs_reg=512, elem_size=DSH)
                    nc.sync.dma_start(
                        out=out[:][512 * c2:512 * (c2 + 1), :].rearrange(
                            "(b a) d -> a b d", a=128),
                        in_=g4[:].rearrange("a (b d) -> a b d", b=4))

                # sel offsets: prefix sums of W_j over the Sel-expanded tiles
                sel_off = []
                acc = 0
                for _, w in sel_key:
                    sel_off.append(acc)
                    acc += w

                def sel_chunk(c2):
                    # out[l] = h[idx[l]] for l-chunk c2 via one-hot matmuls:
                    # psum[a, d] = sum_w sum_q Sel_{j,w}[q, a] *
                    #              h_sb[ktile u_j + w][q, d]
                    g4 = gxsp.tile([128, 4 * DSH], bf16, tag="gxs",
                                   name=f"gsel_{c2}")
                    for jj in range(4):
                        j = 4 * c2 + jj
                        u_j, w_j = sel_key[j - 4 * SEL_START]
                        off = sel_off[j - 4 * SEL_START]
                        ps = selpp.tile([128, DSH], f32, tag="selp",
                                        name=f"psel_{j}")
                        for w in range(w_j):
                            nc.tensor.matmul(
                                out=ps[:],
                                lhsT=sel_sb[:][:, (off + w) * 128:
                                               (off + w + 1) * 128],
                                rhs=h_sb[:][:, (u_j + w) * DSH:
                                            (u_j + w + 1) * DSH],
                                start=(w == 0), stop=(w == w_j - 1))
                        # split the psum->staging copy across ACT and DVE:
                        # halves the per-tile latency on the Sel chain
                        nc.scalar.copy(
                            out=g4[:][:, DSH * jj:DSH * jj + 256],
                            in_=ps[:][:, 0:256])
                        nc.vector.tensor_copy(
                            g4[:][:, DSH * jj + 256:DSH * (jj + 1)],
                            ps[:][:, 256:512])
                    eng = nc.scalar if c2 % 2 == 0 else nc.sync
                    eng.dma_start(
                        out=out[:][512 * c2:512 * (c2 + 1), :].rearrange(
                            "(b a) d -> a b d", a=128),
                        in_=g4[:].rearrange("a (b d) -> a b d", b=4))

                # All scan chunks before all gathers: the DRAM dep tracker
                # is whole-tensor, so interleaving creates false WAR edges
                # (hd write of chunk c+1 waits on the gather of chunk c).
                for c in range(KC):
                    scan_chunk(c)
                for c2 in range(SEL_START):
                    expand_chunk(c2)
                for c2 in range(SEL_START, NLC):
                    sel_chunk(c2)

    nc.compile()
    return nc


def _install_profile_hook():
    """Provide antenv.axon_hooks (missing in this image) so
    run_bass_kernel_spmd(trace=True) can capture NTFF profiles via
    /opt/axon/libaxon_pjrt.so."""
    import sys as _sys
    import types
    import contextlib
    import ctypes

    if "antenv.axon_hooks" in _sys.modules:
        return
    try:
        lib = ctypes.CDLL("/opt/axon/libaxon_pjrt.so")
        if not hasattr(lib, "axon_start_nrt_profile"):
            return
    except OSError:
        return
    lib.axon_start_nrt_profile.argtypes = [
        ctypes.POINTER(ctypes.c_int64), ctypes.c_size_t]
    lib.axon_start_nrt_profile.restype = ctypes.c_int64
    lib.axon_stop_nrt_profile.argtypes = [ctypes.c_char_p]
    lib.axon_stop_nrt_profile.restype = ctypes.c_int64

    @contextlib.contextmanager
    def _hook(output_dir, device_ids):
        import jax
        jax.devices()
        if device_ids:
            ids = (ctypes.c_int64 * len(device_ids))(*device_ids)
            rc = lib.axon_start_nrt_profile(ids, len(device_ids))
        else:
            rc = lib.axon_start_nrt_profile(None, 0)
        if rc != 0:
            raise RuntimeError(f"axon_start_nrt_profile rc={rc}")
        try:
            yield
        finally:
            n = lib.axon_stop_nrt_profile(str(output_dir).encode())
            print(f"profile: {n} file(s) written to {output_dir}",
                  file=sys.stderr)

    m = types.ModuleType("antenv.axon_hooks")
    m.get_axon_ntff_profile_hook = lambda: _hook
    m.set_axon_ntff_profile_hook = lambda h: None
    _sys.modules["antenv.axon_hooks"] = m


def _get_program(kmax, sel_key, hd_hi):
    key = (kmax, sel_key, hd_hi)
    if key not in _progs:
        _progs[key] = _build_program(kmax, sel_key, hd_hi)
    return _progs[key]


def run(inputs, trace=False):
    """Returns (full_output, exec_time_ns or None)."""
    import ml_dtypes
    from concourse.bass_utils import run_bass_kernel_spmd

    bf16 = ml_dtypes.bfloat16
    hidden_states = np.asarray(inputs["hidden_states"], dtype=np.float32)
    boundary_mask = np.asarray(inputs["boundary_mask"]).astype(bool)
    boundary_prob = np.asarray(inputs["boundary_prob"], dtype=np.float32)

    # host index prep (tiny [B, L] arrays)
    p_full = np.clip(boundary_prob[:, :, 1], EPS, 1.0 - EPS)  # (B, L)
    nb = np.cumsum(boundary_mask, axis=1, dtype=np.int64)      # (B, L)
    idx_full = np.maximum(nb - 1, 0).astype(np.int32)          # (B, L)
    Ks = boundary_mask.sum(axis=1)                             # (B,)
    kmax = int(((int(Ks.max()) + 511) // 512) * 512)
    kmax = max(kmax, 512)
    KT = kmax // 128

    pb = np.zeros((B, kmax), np.float32)
    for b in range(B):
        pos = np.flatnonzero(boundary_mask[b])
        pb[b, :len(pos)] = p_full[b, pos]
    arow = (1.0 - pb).reshape(B, 1, kmax)                      # (B, 1, kmax)
    pbcm = np.ascontiguousarray(
        pb.reshape(B, KT, 128).transpose(0, 2, 1))             # (B, 128, KT)
    # dma_gather idx layout: idx i of chunk c at [i%16, 32c + i//16],
    # replicated 8x down partitions.
    idx16 = np.ascontiguousarray(np.tile(
        idx_full.reshape(B, NLC, 32, 16).transpose(0, 3, 1, 2)
        .reshape(B, 16, NLC * 32), (1, 8, 1))).astype(np.int16)  # (B,128,NLC*32)

    # Sel expansion tables for l-chunks >= SEL_START: per l-tile j, the h
    # rows referenced span k-tiles [u_j, u_j + W_j) (shared across rows so
    # one program serves all cores; exact for this dataset, recompiles if
    # the window signature changes).
    sel_tiles = range(4 * SEL_START, NLT)
    sel_key = []
    for j in sel_tiles:
        u_j = int((idx_full[:, 128 * j] // 128).min())
        hi_j = int((idx_full[:, 128 * j + 127] // 128).max())
        sel_key.append((u_j, hi_j - u_j + 1))
    sel_key = tuple(sel_key)
    totW = sum(w for _, w in sel_key)
    sel_np = np.zeros((B, 128, totW * 128), np.float32)
    for b in range(B):
        off = 0
        for j, (u_j, w_j) in zip(sel_tiles, sel_key):
            loc = idx_full[b, 128 * j:128 * (j + 1)] - 128 * u_j
            S = np.zeros((128 * w_j, 128), np.float32)
            S[loc, np.arange(128)] = 1.0
            sel_np[b, :, off * 128:(off + w_j) * 128] = (
                S.reshape(w_j, 128, 128).transpose(1, 0, 2)
                .reshape(128, w_j * 128))
            off += w_j
    sel_np = sel_np.astype(bf16)

    # hd prefix actually referenced by the gather chunks (idx is monotone)
    hd_hi = int(idx_full[:, :512 * SEL_START].max()) + 1
    hd_hi = min(((hd_hi + 511) // 512) * 512, kmax)

    nc = _get_program(kmax, sel_key, hd_hi)
    in_maps = []
    for c in range(NCORES):
        b, h = divmod(c, 2)
        in_maps.append({
            "x": np.ascontiguousarray(
                hidden_states[b, :kmax, h * DSH:(h + 1) * DSH]).astype(bf16),
            "arow": arow[b],
            "pbcm": pbcm[b],
            "idx16": idx16[b],
            "sel": sel_np[b],
            "ident": np.eye(128, dtype=np.float32),
        })
    if trace:
        _install_profile_hook()
    res = run_bass_kernel_spmd(nc, in_maps, list(range(NCORES)), trace=trace)
    outs = res.results
    full = np.empty((B, L, D), np.float32)
    for c in range(NCORES):
        b, h = divmod(c, 2)
        full[b, :, h * DSH:(h + 1) * DSH] = outs[c]["out"].astype(np.float32)
    return full, res.exec_time_ns


def kernel(**inputs) -> np.ndarray:
    out, _ = run(inputs, trace=False)
    return out


# revision 62
# speedup vs baseline: 1.0680x; 1.0680x over previous
"""Trainium2 Bass kernel for nn_DeChunkLayer (ragged EMA de-chunk).

Math (per batch row b):
    p[l]   = clip(boundary_prob[b, l, 1], EPS, 1-EPS)
    nb[l]  = cumsum_l(boundary_mask[b])          (>= 1 since l=0 is a boundary)
    h(k)   = (1-pb[k]) h(k-1) + pb[k] x[k]       (EMA over chunk rank k;
                                                  pb = p at the k-th boundary)
    out[l] = h(nb[l]-1)

Compact-scan + hybrid gather/Sel-matmul expansion (v3, 92.3us vs the
144us fused-scan baseline; mem roofline of the fused design ~94us but
this design moves only ~18MB/core):
  Only ~25% of positions are boundaries, so the EMA has only K ~= 2048
  distinct states and its inputs are x[0:K] read SEQUENTIALLY (the k-th
  EMA step uses x row k, not a gathered row).  Host precomputes the tiny
  per-row index arrays (cumsum nb, gather index idx[l] = nb[l]-1,
  compacted pb[k] = p at the k-th boundary, padded with 0 to K_max so
  padded steps are identity h = 1*h + 0*x).  Device then:
    phase A (compact scan over K_max ~= 2560 instead of 8192):
      per 512-k chunk: ONE sequential x DMA (bf16; HWDGE issue is
      ~0.6us/instruction so DMAs are merged), scale+transpose fused as
      one PE matmul per 128-tile against diag(pb) (frees DVE for the
      scans), DVE tensor_tensor_scan per d-tile with the carry read
      directly from the previous chunk's outT[:,511:512] (no copy), PE
      transpose back, ACT downcast-copy into persistent bf16 h_sb, and
      for k < hd_hi a DMA to DRAM scratch hd (scalar-engine DGE - off
      the sync DGE's in-order stream).
    expansion, split across two engines that run concurrently:
      l-chunks < SEL_START: out[l] = hd[idx[l]] via one dma_gather per
      512-l chunk (SWDGE ucode, int16 idx wrapped 16-per-column and
      replicated 8x down partitions; ~4.6us/call = 994ns fixed +
      ~7ns/descriptor - the same descriptor-gen floor as 4 indirect_dma
      calls, but 4x fewer instructions).  hd only holds rows < hd_hi =
      max idx referenced by these chunks (host bound, ~1024): with the
      whole-tensor DRAM dep tracking, gathers then only wait for the
      first hd_hi/512 scan chunks.
      l-chunks >= SEL_START: one-hot Sel matmuls on the otherwise-idle
      PE: psum[a,d] = sum_w Sel_{j,w}^T @ h_sb[ktile u_j+w], where the
      per-l-tile k-window [u_j, u_j+W_j) is computed on host, shared
      across all 4 rows (so one program serves all 8 cores), and baked
      in at (runtime) compile; Sel one-hots are host-built bf16 uploads.
      One-hot bf16 matmuls reproduce h exactly (no accumulation error).
      PSUM->SBUF copies alternate ACT/DVE; out-DMAs alternate the two
      HWDGE engines (sync/scalar).
  gpsimd runs ONLY mlp-library instructions (identity matrix is a host
  upload, not gpsimd memset/affine_select) so its ucode library loads
  once at t~0 instead of stalling the first gather mid-kernel.
  Output is bf16 on device, upconverted to f32 on host (rel err 2.4e-3,
  budget 2e-2).
  Measured (engine-busy at 93us wall): GpSimd 28us (6 gathers),
  TensorMatrix 54us, DVE 42us (scans 24), ACT 31us, DMA queues ~54%.
  Things measured SLOWER: bf16 scan-out + bf16-PSUM transposes (101us),
  per-l-tile sel out-DMAs (102us), SEL_START=7 (100us), selp=3/po=1
  PSUM rebalance (105us), interleaved scan/gather emission (false WAR
  serialization via whole-tensor DRAM dep tracking, 134us), splitting
  each Sel psum->staging copy into ACT+DVE [128,256] halves (104.5us -
  per-instruction overhead ~0.26us dominates at half size, so both
  engines' serial copy streams got LONGER).

kernel(**inputs) takes FULL inputs, shards over 8 cores (4 batch rows x 2
D-halves), returns FULL (4, 8192, 1024) f32 output.  The Bass program is
compiled on first call per (K_max, sel windows, hd_hi) signature - all
derived from the boundary masks, so any input with the same signature
reuses the cached NEFF and different data simply recompiles.
"""

import os
import sys

import numpy as np

sys.path.insert(0, "/opt/trn_rl_repo")

B, L, D = 4, 8192, 1024
NCORES = 8
DSH = D // 2          # 512 channels per core
NLT = L // 128        # 64 l-tiles of 128
NLC = L // 512        # 16 l-chunks of 512
NDT = DSH // 128      # 4 d-tiles of 128
EPS = 1e-4

_progs = {}  # (kmax, sel_key) -> compiled Bass program

SEL_START = 6  # l-chunks >= this are expanded via PE one-hot matmuls


def _build_program(kmax, sel_key, hd_hi):
    import concourse.bass as bass
    import concourse.mybir as mybir
    from concourse import bacc
    from concourse.tile import TileContext

    f32 = mybir.dt.float32
    bf16 = mybir.dt.bfloat16
    i32 = mybir.dt.int32
    Op = mybir.AluOpType

    KC = kmax // 512      # k-chunks
    KT = kmax // 128      # k-tiles

    nc = bacc.Bacc("TRN2", target_bir_lowering=False, debug=False,
                   num_devices=NCORES)

    i16 = mybir.dt.int16
    totW = sum(w for _, w in sel_key)
    x = nc.declare_dram_parameter("x", [kmax, DSH], bf16, isOutput=False)
    arow_d = nc.declare_dram_parameter("arow", [1, kmax], f32, isOutput=False)
    pbcm_d = nc.declare_dram_parameter("pbcm", [128, KT], f32, isOutput=False)
    sel_d = nc.declare_dram_parameter("sel", [128, totW * 128], bf16,
                                      isOutput=False)
    ident_d = nc.declare_dram_parameter("ident", [128, 128], f32,
                                        isOutput=False)
    # idx16: [128, NLC*32] int16; per l-chunk slice [:, 32c:32c+32] holds the
    # 512 gather indices wrapped 16-per-column (idx i at partition i%16,
    # col i//16), replicated 8x down the partitions for the 8 gpsimd cores.
    idx16_d = nc.declare_dram_parameter("idx16", [128, NLC * 32], i16,
                                        isOutput=False)
    out = nc.declare_dram_parameter("out", [L, DSH], bf16, isOutput=True)

    with TileContext(nc) as tc:
        with (
            tc.tile_pool(name="const", bufs=1) as cpool,
            tc.tile_pool(name="prep", bufs=1) as ppool,
            tc.tile_pool(name="hdp", bufs=1, space="DRAM") as hdp,
        ):
            # identity uploaded from host: keeps gpsimd free of non-mlp
            # library instructions (its ucode library reload would other-
            # wise stall the first dma_gather ~10us mid-kernel)
            ident = cpool.tile([128, 128], f32, tag="ident")
            nc.sync.dma_start(out=ident[:], in_=ident_d[:])
            identb = cpool.tile([128, 128], bf16, tag="identb")
            nc.vector.tensor_copy(identb[:], ident[:])
            ones1 = cpool.tile([1, 128], f32, tag="ones1")
            nc.vector.memset(ones1[:], 1.0)

            arow = ppool.tile([1, kmax], f32, tag="arow")
            nc.sync.dma_start(out=arow[:], in_=arow_d[:])
            pbcm = ppool.tile([128, KT], f32, tag="pbcm")
            nc.sync.dma_start(out=pbcm[:], in_=pbcm_d[:])
            idx16 = ppool.tile([128, NLC * 32], i16, tag="idx16")
            nc.sync.dma_start(out=idx16[:], in_=idx16_d[:])
            sel_sb = ppool.tile([128, totW * 128], bf16, tag="sel_sb")
            nc.scalar.dma_start(out=sel_sb[:], in_=sel_d[:])
            # persistent k-major h (bf16) — rhs of the Sel expansion matmuls
            h_sb = ppool.tile([128, KT * 512], bf16, tag="h_sb")
            # diag(pb) per k-tile: fuses the bn = pb*x scale into the PE
            # transpose (regular matmul with a diagonal rhs), freeing DVE.
            diag = ppool.tile([128, KT * 128], bf16, tag="diag")
            for t in range(KT):
                nc.vector.tensor_scalar_mul(
                    diag[:][:, 128 * t:128 * (t + 1)], identb[:],
                    pbcm[:][:, t:t + 1])

            # Only the hd prefix the gather chunks reference (host-verified
            # bound on idx over l < 512*SEL_START) goes to DRAM; gathers
            # then wait only on the first hd_hi/512 scan chunks.
            hd = hdp.tile([hd_hi, DSH], bf16, tag="hd", name="hd")

            # chunk-to-chunk scan carries: previous chunk's outT last
            # column is read directly as `initial` (outT pool is 8 deep =
            # 2 chunks, so the slice is alive when the next scan starts)
            prev_outTs = [None] * NDT

            with (
                tc.tile_pool(name="xg", bufs=4) as xgp,
                tc.tile_pool(name="bt", bufs=4, space="PSUM") as btp,
                tc.tile_pool(name="asb", bufs=2) as asbp,
                tc.tile_pool(name="outT", bufs=8) as otp,
                tc.tile_pool(name="po", bufs=2, space="PSUM") as pop,
                tc.tile_pool(name="selp", bufs=2, space="PSUM") as selpp,
                tc.tile_pool(name="gx", bufs=4) as gxp,
                tc.tile_pool(name="gxs", bufs=4) as gxsp,
            ):
                def scan_chunk(c):
                    # broadcast a = (1-pb)[chunk] to 128 partitions
                    apm = pop.tile([128, 512], f32, tag="po",
                                   name=f"apm_{c}")
                    nc.tensor.matmul(
                        out=apm[:], lhsT=ones1[:],
                        rhs=arow[:][0:1, 512 * c:512 * (c + 1)],
                        start=True, stop=True)
                    a_sb = asbp.tile([128, 512], f32, tag="a_sb",
                                     name=f"asb_{c}")
                    nc.scalar.copy(out=a_sb[:], in_=apm[:])

                    # sequential x rows for this chunk (bf16), one DMA
                    # (sequencer issue costs ~0.6us per DMA instruction)
                    xg4 = xgp.tile([128, 4 * DSH], bf16, tag="xg",
                                   name=f"xg_{c}")
                    nc.sync.dma_start(
                        out=xg4[:].rearrange("p (b d) -> p b d", b=4),
                        in_=x[:][512 * c:512 * (c + 1), :].rearrange(
                            "(b p) d -> p b d", p=128))

                    bts = [btp.tile([128, 512], f32, tag="bt",
                                    name=f"bt{t}_{c}")
                           for t in range(NDT)]
                    for kt in range(4):
                        t_col = 4 * c + kt
                        for t in range(NDT):
                            # bts[t][d, k'] = sum_k x[k, d] * diag[k, k']
                            #              = x[k', d] * pb[k']
                            nc.tensor.matmul(
                                out=bts[t][:][:, 128 * kt:128 * (kt + 1)],
                                lhsT=xg4[:][:, DSH * kt + 128 * t:
                                            DSH * kt + 128 * (t + 1)],
                                rhs=diag[:][:, 128 * t_col:128 * (t_col + 1)],
                                start=True, stop=True)

                    outTs = [otp.tile([128, 512], f32, tag=f"outT{t}",
                                      name=f"outT{t}_{c}")
                             for t in range(NDT)]
                    for t in range(NDT):
                        nc.vector.tensor_tensor_scan(
                            out=outTs[t][:], data0=a_sb[:], data1=bts[t][:],
                            initial=(0.0 if c == 0
                                     else prev_outTs[t][:][:, 511:512]),
                            op0=Op.mult, op1=Op.add)
                        prev_outTs[t] = outTs[t]

                    # transpose back to [k, d] rows, downcast to bf16, store
                    # into the persistent h_sb (Sel matmul rhs + hd source)
                    for kt in range(4):
                        po = pop.tile([128, DSH], f32, tag="po",
                                      name=f"po_{c}_{kt}")
                        for t in range(NDT):
                            nc.tensor.transpose(
                                out=po[:][:, 128 * t:128 * (t + 1)],
                                in_=outTs[t][:][:, 128 * kt:128 * (kt + 1)],
                                identity=ident[:])
                        nc.scalar.copy(
                            out=h_sb[:][:, (4 * c + kt) * DSH:
                                        (4 * c + kt + 1) * DSH], in_=po[:])
                    # scalar-engine DGE: keeps hd writes off the sync
                    # sequencer's in-order stream (x loads / out writes)
                    if 512 * c < hd_hi:
                        nc.scalar.dma_start(
                            out=hd[:][512 * c:512 * (c + 1), :].rearrange(
                                "(b a) d -> a b d", a=128),
                            in_=h_sb[:][:, 4 * c * DSH:4 * (c + 1) * DSH]
                            .rearrange("a (b d) -> a b d", b=4))

                def expand_chunk(c2):
                    # h rows needed by l-chunk c2 are < 512*(c2+1) since
                    # idx[l] <= l; slice the gather src so the DRAM dep
                    # tracker only orders against already-written chunks.
                    hi = min(512 * (c2 + 1), hd_hi)
                    g4 = gxp.tile([128, 4 * DSH], bf16, tag="gx",
                                  name=f"gx_{c2}")
                    nc.gpsimd.dma_gather(
                        out_ap=g4[:].rearrange("a (b d) -> a b d", b=4),
                        in_ap=hd[:][0:hi, :],
                        idxs_ap=idx16[:][:, 32 * c2:32 * (c2 + 1)],
                        num_idxs=512, num_idxs_reg=512, elem_size=DSH)
                    nc.sync.dma_start(
                        out=out[:][512 * c2:512 * (c2 + 1), :].rearrange(
                            "(b a) d -> a b d", a=128),
                        in_=g4[:].rearrange("a (b d) -> a b d", b=4))

                # sel offsets: prefix sums of W_j over the Sel-expanded tiles
                sel_off = []
                acc = 0
                for _, w in sel_key:
                    sel_off.append(acc)
                    acc += w

                def sel_chunk(c2):
                    # out[l] = h[idx[l]] for l-chunk c2 via one-hot matmuls:
                    # psum[a, d] = sum_w sum_q Sel_{j,w}[q, a] *
                    #              h_sb[ktile u_j + w][q, d]
                    g4 = gxsp.tile([128, 4 * DSH], bf16, tag="gxs",
                                   name=f"gsel_{c2}")
                    for jj in range(4):
                        j = 4 * c2 + jj
                        u_j, w_j = sel_key[j - 4 * SEL_START]
                        off = sel_off[j - 4 * SEL_START]
                        ps = selpp.tile([128, DSH], f32, tag="selp",
                                        name=f"psel_{j}")
                        for w in range(w_j):
                            nc.tensor.matmul(
                                out=ps[:],
                                lhsT=sel_sb[:][:, (off + w) * 128:
                                               (off + w + 1) * 128],
                                rhs=h_sb[:][:, (u_j + w) * DSH:
                                            (u_j + w + 1) * DSH],
                                start=(w == 0), stop=(w == w_j - 1))
                        dst = g4[:][:, DSH * jj:DSH * (jj + 1)]
                        if jj % 2 == 0:
                            nc.scalar.copy(out=dst, in_=ps[:])
                        else:
                            nc.vector.tensor_copy(dst, ps[:])
                    eng = nc.scalar if c2 % 2 == 0 else nc.sync
                    eng.dma_start(
                        out=out[:][512 * c2:512 * (c2 + 1), :].rearrange(
                            "(b a) d -> a b d", a=128),
                        in_=g4[:].rearrange("a (b d) -> a b d", b=4))

                # All scan chunks before all gathers: the DRAM dep tracker
                # is whole-tensor, so interleaving creates false WAR edges
                # (hd write of chunk c+1 waits on the gather of chunk c).
                for c in range(KC):
                    scan_chunk(c)
                for c2 in range(SEL_START):
                    expand_chunk(c2)
                for c2 in range(SEL_START, NLC):
                    sel_chunk(c2)

    nc.compile()
    return nc


def _install_profile_hook():
    """Provide antenv.axon_hooks (missing in this image) so
    run_bass_kernel_spmd(trace=True) can capture NTFF profiles via
    /opt/axon/libaxon_pjrt.so."""
    import sys as _sys
    import types
    import contextlib
    import ctypes

    if "antenv.axon_hooks" in _sys.modules:
        return
    try:
        lib = ctypes.CDLL("/opt/axon/libaxon_pjrt.so")
        if not hasattr(lib, "axon_start_nrt_profile"):
            return
    except OSError:
        return
    lib.axon_start_nrt_profile.argtypes = [
        ctypes.POINTER(ctypes.c_int64), ctypes.c_size_t]
    lib.axon_start_nrt_profile.restype = ctypes.c_int64
    lib.axon_stop_nrt_profile.argtypes = [ctypes.c_char_p]
    lib.axon_stop_nrt_profile.restype = ctypes.c_int64

    @contextlib.contextmanager
    def _hook(output_dir, device_ids):
        import jax
        jax.devices()
        if device_ids:
            ids = (ctypes.c_int64 * len(device_ids))(*device_ids)
            rc = lib.axon_start_nrt_profile(ids, len(device_ids))
        else:
            rc = lib.axon_start_nrt_profile(None, 0)
        if rc != 0:
            raise RuntimeError(f"axon_start_nrt_profile rc={rc}")
        try:
            yield
        finally:
            n = lib.axon_stop_nrt_profile(str(output_dir).encode())
            print(f"profile: {n} file(s) written to {output_dir}",
                  file=sys.stderr)

    m = types.ModuleType("antenv.axon_hooks")
    m.get_axon_ntff_profile_hook = lambda: _hook
    m.set_axon_ntff_profile_hook = lambda h: None
    _sys.modules["antenv.axon_hooks"] = m


def _get_program(kmax, sel_key, hd_hi):
    key = (kmax, sel_key, hd_hi)
    if key not in _progs:
        _progs[key] = _build_program(kmax, sel_key, hd_hi)
    return _progs[key]


def run(inputs, trace=False):
    """Returns (full_output, exec_time_ns or None)."""
    import ml_dtypes
    from concourse.bass_utils import run_bass_kernel_spmd

    bf16 = ml_dtypes.bfloat16
    hidden_states = np.asarray(inputs["hidden_states"], dtype=np.float32)
    boundary_mask = np.asarray(inputs["boundary_mask"]).astype(bool)
    boundary_prob = np.asarray(inputs["boundary_prob"], dtype=np.float32)

    # host index prep (tiny [B, L] arrays)
    p_full = np.clip(boundary_prob[:, :, 1], EPS, 1.0 - EPS)  # (B, L)
    nb = np.cumsum(boundary_mask, axis=1, dtype=np.int64)      # (B, L)
    idx_full = np.maximum(nb - 1, 0).astype(np.int32)          # (B, L)
    Ks = boundary_mask.sum(axis=1)                             # (B,)
    kmax = int(((int(Ks.max()) + 511) // 512) * 512)
    kmax = max(kmax, 512)
    KT = kmax // 128

    pb = np.zeros((B, kmax), np.float32)
    for b in range(B):
        pos = np.flatnonzero(boundary_mask[b])
        pb[b, :len(pos)] = p_full[b, pos]
    arow = (1.0 - pb).reshape(B, 1, kmax)                      # (B, 1, kmax)
    pbcm = np.ascontiguousarray(
        pb.reshape(B, KT, 128).transpose(0, 2, 1))             # (B, 128, KT)
    # dma_gather idx layout: idx i of chunk c at [i%16, 32c + i//16],
    # replicated 8x down partitions.
    idx16 = np.ascontiguousarray(np.tile(
        idx_full.reshape(B, NLC, 32, 16).transpose(0, 3, 1, 2)
        .reshape(B, 16, NLC * 32), (1, 8, 1))).astype(np.int16)  # (B,128,NLC*32)

    # Sel expansion tables for l-chunks >= SEL_START: per l-tile j, the h
    # rows referenced span k-tiles [u_j, u_j + W_j) (shared across rows so
    # one program serves all cores; exact for this dataset, recompiles if
    # the window signature changes).
    sel_tiles = range(4 * SEL_START, NLT)
    sel_key = []
    for j in sel_tiles:
        u_j = int((idx_full[:, 128 * j] // 128).min())
        hi_j = int((idx_full[:, 128 * j + 127] // 128).max())
        sel_key.append((u_j, hi_j - u_j + 1))
    sel_key = tuple(sel_key)
    totW = sum(w for _, w in sel_key)
    sel_np = np.zeros((B, 128, totW * 128), np.float32)
    for b in range(B):
        off = 0
        for j, (u_j, w_j) in zip(sel_tiles, sel_key):
            loc = idx_full[b, 128 * j:128 * (j + 1)] - 128 * u_j
            S = np.zeros((128 * w_j, 128), np.float32)
            S[loc, np.arange(128)] = 1.0
            sel_np[b, :, off * 128:(off + w_j) * 128] = (
                S.reshape(w_j, 128, 128).transpose(1, 0, 2)
                .reshape(128, w_j * 128))
            off += w_j
    sel_np = sel_np.astype(bf16)

    # hd prefix actually referenced by the gather chunks (idx is monotone)
    hd_hi = int(idx_full[:, :512 * SEL_START].max()) + 1
    hd_hi = min(((hd_hi + 511) // 512) * 512, kmax)

    nc = _get_program(kmax, sel_key, hd_hi)
    in_maps = []
    for c in range(NCORES):
        b, h = divmod(c, 2)
        in_maps.append({
            "x": np.ascontiguousarray(
                hidden_states[b, :kmax, h * DSH:(h + 1) * DSH]).astype(bf16),
            "arow": arow[b],
            "pbcm": pbcm[b],
            "idx16": idx16[b],
            "sel": sel_np[b],
            "ident": np.eye(128, dtype=np.float32),
        })
    if trace:
        _install_profile_hook()
    res = run_bass_kernel_spmd(nc, in_maps, list(range(NCORES)), trace=trace)
    outs = res.results
    full = np.empty((B, L, D), np.float32)
    for c in range(NCORES):
        b, h = divmod(c, 2)
        full[b, :, h * DSH:(h + 1) * DSH] = outs[c]["out"].astype(np.float32)
    return full, res.exec_time_ns


def kernel(**inputs) -> np.ndarray:
    out, _ = run(inputs, trace=False)
    return out


# revision 63
# speedup vs baseline: 1.1112x; 1.0404x over previous
"""Trainium2 Bass kernel for nn_DeChunkLayer (ragged EMA de-chunk).

Math (per batch row b):
    p[l]   = clip(boundary_prob[b, l, 1], EPS, 1-EPS)
    nb[l]  = cumsum_l(boundary_mask[b])          (>= 1 since l=0 is a boundary)
    h(k)   = (1-pb[k]) h(k-1) + pb[k] x[k]       (EMA over chunk rank k;
                                                  pb = p at the k-th boundary)
    out[l] = h(nb[l]-1)

Compact-scan + hybrid gather/Sel-matmul expansion (v3, 92.3us vs the
144us fused-scan baseline; mem roofline of the fused design ~94us but
this design moves only ~18MB/core):
  Only ~25% of positions are boundaries, so the EMA has only K ~= 2048
  distinct states and its inputs are x[0:K] read SEQUENTIALLY (the k-th
  EMA step uses x row k, not a gathered row).  Host precomputes the tiny
  per-row index arrays (cumsum nb, gather index idx[l] = nb[l]-1,
  compacted pb[k] = p at the k-th boundary, padded with 0 to K_max so
  padded steps are identity h = 1*h + 0*x).  Device then:
    phase A (compact scan over K_max ~= 2560 instead of 8192):
      per 512-k chunk: ONE sequential x DMA (bf16; HWDGE issue is
      ~0.6us/instruction so DMAs are merged), scale+transpose fused as
      one PE matmul per 128-tile against diag(pb) (frees DVE for the
      scans), DVE tensor_tensor_scan per d-tile with the carry read
      directly from the previous chunk's outT[:,511:512] (no copy), PE
      transpose back, ACT downcast-copy into persistent bf16 h_sb, and
      for k < hd_hi a DMA to DRAM scratch hd (scalar-engine DGE - off
      the sync DGE's in-order stream).
    expansion, split across two engines that run concurrently:
      l-chunks < SEL_START: out[l] = hd[idx[l]] via one dma_gather per
      512-l chunk (SWDGE ucode, int16 idx wrapped 16-per-column and
      replicated 8x down partitions; ~4.6us/call = 994ns fixed +
      ~7ns/descriptor - the same descriptor-gen floor as 4 indirect_dma
      calls, but 4x fewer instructions).  hd only holds rows < hd_hi =
      max idx referenced by these chunks (host bound, ~1024): with the
      whole-tensor DRAM dep tracking, gathers then only wait for the
      first hd_hi/512 scan chunks.
      l-chunks >= SEL_START: one-hot Sel matmuls on the otherwise-idle
      PE: psum[a,d] = sum_w Sel_{j,w}^T @ h_sb[ktile u_j+w], where the
      per-l-tile k-window [u_j, u_j+W_j) is computed on host, shared
      across all 4 rows (so one program serves all 8 cores), and baked
      in at (runtime) compile; Sel one-hots are host-built bf16 uploads.
      One-hot bf16 matmuls reproduce h exactly (no accumulation error).
      PSUM->SBUF copies alternate ACT/DVE; out-DMAs alternate the two
      HWDGE engines (sync/scalar).
  gpsimd runs ONLY mlp-library instructions (identity matrix is a host
  upload, not gpsimd memset/affine_select) so its ucode library loads
  once at t~0 instead of stalling the first gather mid-kernel.
  Output is bf16 on device, upconverted to f32 on host (rel err 2.4e-3,
  budget 2e-2).
  Measured (engine-busy at 93us wall): GpSimd 28us (6 gathers),
  TensorMatrix 54us, DVE 42us (scans 24), ACT 31us, DMA queues ~54%.
  Things measured SLOWER: bf16 scan-out + bf16-PSUM transposes (101us),
  per-l-tile sel out-DMAs (102us), SEL_START=7 (100us), selp=3/po=1
  PSUM rebalance (105us), interleaved scan/gather emission (false WAR
  serialization via whole-tensor DRAM dep tracking, 134us), splitting
  each Sel psum->staging copy into ACT+DVE [128,256] halves (104.5us -
  per-instruction overhead ~0.26us dominates at half size, so both
  engines' serial copy streams got LONGER).

kernel(**inputs) takes FULL inputs, shards over 8 cores (4 batch rows x 2
D-halves), returns FULL (4, 8192, 1024) f32 output.  The Bass program is
compiled on first call per (K_max, sel windows, hd_hi) signature - all
derived from the boundary masks, so any input with the same signature
reuses the cached NEFF and different data simply recompiles.
"""

import os
import sys

import numpy as np

sys.path.insert(0, "/opt/trn_rl_repo")

B, L, D = 4, 8192, 1024
NCORES = 8
DSH = D // 2          # 512 channels per core
NLT = L // 128        # 64 l-tiles of 128
NLC = L // 512        # 16 l-chunks of 512
NDT = DSH // 128      # 4 d-tiles of 128
EPS = 1e-4

_progs = {}  # (kmax, sel_key) -> compiled Bass program

SEL_START = 6  # l-chunks >= this are expanded via PE one-hot matmuls


def _build_program(kmax, sel_key, hd_hi):
    import concourse.bass as bass
    import concourse.mybir as mybir
    from concourse import bacc
    from concourse.tile import TileContext

    f32 = mybir.dt.float32
    bf16 = mybir.dt.bfloat16
    i32 = mybir.dt.int32
    Op = mybir.AluOpType

    KC = kmax // 512      # k-chunks
    KT = kmax // 128      # k-tiles

    nc = bacc.Bacc("TRN2", target_bir_lowering=False, debug=False,
                   num_devices=NCORES)

    i16 = mybir.dt.int16
    totW = sum(w for _, w in sel_key)
    x = nc.declare_dram_parameter("x", [kmax, DSH], bf16, isOutput=False)
    arow_d = nc.declare_dram_parameter("arow", [1, kmax], f32, isOutput=False)
    pbcm_d = nc.declare_dram_parameter("pbcm", [128, KT], f32, isOutput=False)
    sel_d = nc.declare_dram_parameter("sel", [128, totW * 128], bf16,
                                      isOutput=False)
    ident_d = nc.declare_dram_parameter("ident", [128, 128], f32,
                                        isOutput=False)
    # idx16: [128, NLC*32] int16; per l-chunk slice [:, 32c:32c+32] holds the
    # 512 gather indices wrapped 16-per-column (idx i at partition i%16,
    # col i//16), replicated 8x down the partitions for the 8 gpsimd cores.
    idx16_d = nc.declare_dram_parameter("idx16", [128, NLC * 32], i16,
                                        isOutput=False)
    out = nc.declare_dram_parameter("out", [L, DSH], bf16, isOutput=True)

    with TileContext(nc) as tc:
        with (
            tc.tile_pool(name="const", bufs=1) as cpool,
            tc.tile_pool(name="prep", bufs=1) as ppool,
            tc.tile_pool(name="hdp", bufs=1, space="DRAM") as hdp,
        ):
            # identity uploaded from host: keeps gpsimd free of non-mlp
            # library instructions (its ucode library reload would other-
            # wise stall the first dma_gather ~10us mid-kernel)
            ident = cpool.tile([128, 128], f32, tag="ident")
            nc.sync.dma_start(out=ident[:], in_=ident_d[:])
            identb = cpool.tile([128, 128], bf16, tag="identb")
            nc.vector.tensor_copy(identb[:], ident[:])
            ones1 = cpool.tile([1, 128], f32, tag="ones1")
            nc.vector.memset(ones1[:], 1.0)

            arow = ppool.tile([1, kmax], f32, tag="arow")
            nc.sync.dma_start(out=arow[:], in_=arow_d[:])
            pbcm = ppool.tile([128, KT], f32, tag="pbcm")
            nc.sync.dma_start(out=pbcm[:], in_=pbcm_d[:])
            idx16 = ppool.tile([128, NLC * 32], i16, tag="idx16")
            nc.sync.dma_start(out=idx16[:], in_=idx16_d[:])
            sel_sb = ppool.tile([128, totW * 128], bf16, tag="sel_sb")
            nc.scalar.dma_start(out=sel_sb[:], in_=sel_d[:])
            # persistent k-major h (bf16) — rhs of the Sel expansion matmuls
            h_sb = ppool.tile([128, KT * 512], bf16, tag="h_sb")
            # diag(pb) per k-tile: fuses the bn = pb*x scale into the PE
            # transpose (regular matmul with a diagonal rhs), freeing DVE.
            diag = ppool.tile([128, KT * 128], bf16, tag="diag")
            for t in range(KT):
                nc.vector.tensor_scalar_mul(
                    diag[:][:, 128 * t:128 * (t + 1)], identb[:],
                    pbcm[:][:, t:t + 1])

            # Only the hd prefix the gather chunks reference (host-verified
            # bound on idx over l < 512*SEL_START) goes to DRAM; gathers
            # then wait only on the first hd_hi/512 scan chunks.
            hd = hdp.tile([hd_hi, DSH], bf16, tag="hd", name="hd")

            # chunk-to-chunk scan carries: previous chunk's outT last
            # column is read directly as `initial` (outT pool is 8 deep =
            # 2 chunks, so the slice is alive when the next scan starts)
            prev_outTs = [None] * NDT

            with (
                tc.tile_pool(name="xg", bufs=4) as xgp,
                tc.tile_pool(name="bt", bufs=4, space="PSUM") as btp,
                tc.tile_pool(name="asb", bufs=2) as asbp,
                tc.tile_pool(name="outT", bufs=8) as otp,
                tc.tile_pool(name="po", bufs=2, space="PSUM") as pop,
                tc.tile_pool(name="selp", bufs=2, space="PSUM") as selpp,
                tc.tile_pool(name="gx", bufs=4) as gxp,
                tc.tile_pool(name="gxs", bufs=4) as gxsp,
            ):
                def scan_chunk(c):
                    # broadcast a = (1-pb)[chunk] to 128 partitions
                    apm = pop.tile([128, 512], f32, tag="po",
                                   name=f"apm_{c}")
                    nc.tensor.matmul(
                        out=apm[:], lhsT=ones1[:],
                        rhs=arow[:][0:1, 512 * c:512 * (c + 1)],
                        start=True, stop=True)
                    a_sb = asbp.tile([128, 512], f32, tag="a_sb",
                                     name=f"asb_{c}")
                    nc.scalar.copy(out=a_sb[:], in_=apm[:])

                    # sequential x rows for this chunk (bf16), one DMA
                    # (sequencer issue costs ~0.6us per DMA instruction)
                    xg4 = xgp.tile([128, 4 * DSH], bf16, tag="xg",
                                   name=f"xg_{c}")
                    nc.sync.dma_start(
                        out=xg4[:].rearrange("p (b d) -> p b d", b=4),
                        in_=x[:][512 * c:512 * (c + 1), :].rearrange(
                            "(b p) d -> p b d", p=128))

                    bts = [btp.tile([128, 512], f32, tag="bt",
                                    name=f"bt{t}_{c}")
                           for t in range(NDT)]
                    for kt in range(4):
                        t_col = 4 * c + kt
                        for t in range(NDT):
                            # bts[t][d, k'] = sum_k x[k, d] * diag[k, k']
                            #              = x[k', d] * pb[k']
                            nc.tensor.matmul(
                                out=bts[t][:][:, 128 * kt:128 * (kt + 1)],
                                lhsT=xg4[:][:, DSH * kt + 128 * t:
                                            DSH * kt + 128 * (t + 1)],
                                rhs=diag[:][:, 128 * t_col:128 * (t_col + 1)],
                                start=True, stop=True)

                    outTs = [otp.tile([128, 512], f32, tag=f"outT{t}",
                                      name=f"outT{t}_{c}")
                             for t in range(NDT)]
                    for t in range(NDT):
                        nc.vector.tensor_tensor_scan(
                            out=outTs[t][:], data0=a_sb[:], data1=bts[t][:],
                            initial=(0.0 if c == 0
                                     else prev_outTs[t][:][:, 511:512]),
                            op0=Op.mult, op1=Op.add)
                        prev_outTs[t] = outTs[t]

                    # transpose back to [k, d] rows, downcast to bf16, store
                    # into the persistent h_sb (Sel matmul rhs + hd source)
                    for kt in range(4):
                        po = pop.tile([128, DSH], f32, tag="po",
                                      name=f"po_{c}_{kt}")
                        for t in range(NDT):
                            nc.tensor.transpose(
                                out=po[:][:, 128 * t:128 * (t + 1)],
                                in_=outTs[t][:][:, 128 * kt:128 * (kt + 1)],
                                identity=ident[:])
                        nc.scalar.copy(
                            out=h_sb[:][:, (4 * c + kt) * DSH:
                                        (4 * c + kt + 1) * DSH], in_=po[:])
                    # scalar-engine DGE: keeps hd writes off the sync
                    # sequencer's in-order stream (x loads / out writes)
                    if 512 * c < hd_hi:
                        nc.scalar.dma_start(
                            out=hd[:][512 * c:512 * (c + 1), :].rearrange(
                                "(b a) d -> a b d", a=128),
                            in_=h_sb[:][:, 4 * c * DSH:4 * (c + 1) * DSH]
                            .rearrange("a (b d) -> a b d", b=4))

                def expand_chunk(c2):
                    # h rows needed by l-chunk c2 are < 512*(c2+1) since
                    # idx[l] <= l; slice the gather src so the DRAM dep
                    # tracker only orders against already-written chunks.
                    hi = min(512 * (c2 + 1), hd_hi)
                    g4 = gxp.tile([128, 4 * DSH], bf16, tag="gx",
                                  name=f"gx_{c2}")
                    nc.gpsimd.dma_gather(
                        out_ap=g4[:].rearrange("a (b d) -> a b d", b=4),
                        in_ap=hd[:][0:hi, :],
                        idxs_ap=idx16[:][:, 32 * c2:32 * (c2 + 1)],
                        num_idxs=512, num_idxs_reg=512, elem_size=DSH)
                    nc.sync.dma_start(
                        out=out[:][512 * c2:512 * (c2 + 1), :].rearrange(
                            "(b a) d -> a b d", a=128),
                        in_=g4[:].rearrange("a (b d) -> a b d", b=4))

                # sel offsets: prefix sums of W_j over the Sel-expanded tiles
                sel_off = []
                acc = 0
                for _, w in sel_key:
                    sel_off.append(acc)
                    acc += w

                def sel_chunk(c2):
                    # out[l] = h[idx[l]] for l-chunk c2 via one-hot matmuls:
                    # psum[a, d] = sum_w sum_q Sel_{j,w}[q, a] *
                    #              h_sb[ktile u_j + w][q, d]
                    g4 = gxsp.tile([128, 4 * DSH], bf16, tag="gxs",
                                   name=f"gsel_{c2}")
                    for jj in range(4):
                        j = 4 * c2 + jj
                        u_j, w_j = sel_key[j - 4 * SEL_START]
                        off = sel_off[j - 4 * SEL_START]
                        # alternate psum pools: pop's 2 banks are idle
                        # after phase A, giving Sel a 4-deep psum pipeline
                        pool, ptag = ((selpp, "selp") if jj % 2 == 0
                                      else (pop, "po"))
                        ps = pool.tile([128, DSH], f32, tag=ptag,
                                       name=f"psel_{j}")
                        for w in range(w_j):
                            nc.tensor.matmul(
                                out=ps[:],
                                lhsT=sel_sb[:][:, (off + w) * 128:
                                               (off + w + 1) * 128],
                                rhs=h_sb[:][:, (u_j + w) * DSH:
                                            (u_j + w + 1) * DSH],
                                start=(w == 0), stop=(w == w_j - 1))
                        dst = g4[:][:, DSH * jj:DSH * (jj + 1)]
                        if jj % 2 == 0:
                            nc.scalar.copy(out=dst, in_=ps[:])
                        else:
                            nc.vector.tensor_copy(dst, ps[:])
                    eng = nc.scalar if c2 % 2 == 0 else nc.sync
                    eng.dma_start(
                        out=out[:][512 * c2:512 * (c2 + 1), :].rearrange(
                            "(b a) d -> a b d", a=128),
                        in_=g4[:].rearrange("a (b d) -> a b d", b=4))

                # All scan chunks before all gathers: the DRAM dep tracker
                # is whole-tensor, so interleaving creates false WAR edges
                # (hd write of chunk c+1 waits on the gather of chunk c).
                for c in range(KC):
                    scan_chunk(c)
                for c2 in range(SEL_START):
                    expand_chunk(c2)
                for c2 in range(SEL_START, NLC):
                    sel_chunk(c2)

    nc.compile()
    return nc


def _install_profile_hook():
    """Provide antenv.axon_hooks (missing in this image) so
    run_bass_kernel_spmd(trace=True) can capture NTFF profiles via
    /opt/axon/libaxon_pjrt.so."""
    import sys as _sys
    import types
    import contextlib
    import ctypes

    if "antenv.axon_hooks" in _sys.modules:
        return
    try:
        lib = ctypes.CDLL("/opt/axon/libaxon_pjrt.so")
        if not hasattr(lib, "axon_start_nrt_profile"):
            return
    except OSError:
        return
    lib.axon_start_nrt_profile.argtypes = [
        ctypes.POINTER(ctypes.c_int64), ctypes.c_size_t]
    lib.axon_start_nrt_profile.restype = ctypes.c_int64
    lib.axon_stop_nrt_profile.argtypes = [ctypes.c_char_p]
    lib.axon_stop_nrt_profile.restype = ctypes.c_int64

    @contextlib.contextmanager
    def _hook(output_dir, device_ids):
        import jax
        jax.devices()
        if device_ids:
            ids = (ctypes.c_int64 * len(device_ids))(*device_ids)
            rc = lib.axon_start_nrt_profile(ids, len(device_ids))
        else:
            rc = lib.axon_start_nrt_profile(None, 0)
        if rc != 0:
            raise RuntimeError(f"axon_start_nrt_profile rc={rc}")
        try:
            yield
        finally:
            n = lib.axon_stop_nrt_profile(str(output_dir).encode())
            print(f"profile: {n} file(s) written to {output_dir}",
                  file=sys.stderr)

    m = types.ModuleType("antenv.axon_hooks")
    m.get_axon_ntff_profile_hook = lambda: _hook
    m.set_axon_ntff_profile_hook = lambda h: None
    _sys.modules["antenv.axon_hooks"] = m


def _get_program(kmax, sel_key, hd_hi):
    key = (kmax, sel_key, hd_hi)
    if key not in _progs:
        _progs[key] = _build_program(kmax, sel_key, hd_hi)
    return _progs[key]


def run(inputs, trace=False):
    """Returns (full_output, exec_time_ns or None)."""
    import ml_dtypes
    from concourse.bass_utils import run_bass_kernel_spmd

    bf16 = ml_dtypes.bfloat16
    hidden_states = np.asarray(inputs["hidden_states"], dtype=np.float32)
    boundary_mask = np.asarray(inputs["boundary_mask"]).astype(bool)
    boundary_prob = np.asarray(inputs["boundary_prob"], dtype=np.float32)

    # host index prep (tiny [B, L] arrays)
    p_full = np.clip(boundary_prob[:, :, 1], EPS, 1.0 - EPS)  # (B, L)
    nb = np.cumsum(boundary_mask, axis=1, dtype=np.int64)      # (B, L)
    idx_full = np.maximum(nb - 1, 0).astype(np.int32)          # (B, L)
    Ks = boundary_mask.sum(axis=1)                             # (B,)
    kmax = int(((int(Ks.max()) + 511) // 512) * 512)
    kmax = max(kmax, 512)
    KT = kmax // 128

    pb = np.zeros((B, kmax), np.float32)
    for b in range(B):
        pos = np.flatnonzero(boundary_mask[b])
        pb[b, :len(pos)] = p_full[b, pos]
    arow = (1.0 - pb).reshape(B, 1, kmax)                      # (B, 1, kmax)
    pbcm = np.ascontiguousarray(
        pb.reshape(B, KT, 128).transpose(0, 2, 1))             # (B, 128, KT)
    # dma_gather idx layout: idx i of chunk c at [i%16, 32c + i//16],
    # replicated 8x down partitions.
    idx16 = np.ascontiguousarray(np.tile(
        idx_full.reshape(B, NLC, 32, 16).transpose(0, 3, 1, 2)
        .reshape(B, 16, NLC * 32), (1, 8, 1))).astype(np.int16)  # (B,128,NLC*32)

    # Sel expansion tables for l-chunks >= SEL_START: per l-tile j, the h
    # rows referenced span k-tiles [u_j, u_j + W_j) (shared across rows so
    # one program serves all cores; exact for this dataset, recompiles if
    # the window signature changes).
    sel_tiles = range(4 * SEL_START, NLT)
    sel_key = []
    for j in sel_tiles:
        u_j = int((idx_full[:, 128 * j] // 128).min())
        hi_j = int((idx_full[:, 128 * j + 127] // 128).max())
        sel_key.append((u_j, hi_j - u_j + 1))
    sel_key = tuple(sel_key)
    totW = sum(w for _, w in sel_key)
    sel_np = np.zeros((B, 128, totW * 128), np.float32)
    for b in range(B):
        off = 0
        for j, (u_j, w_j) in zip(sel_tiles, sel_key):
            loc = idx_full[b, 128 * j:128 * (j + 1)] - 128 * u_j
            S = np.zeros((128 * w_j, 128), np.float32)
            S[loc, np.arange(128)] = 1.0
            sel_np[b, :, off * 128:(off + w_j) * 128] = (
                S.reshape(w_j, 128, 128).transpose(1, 0, 2)
                .reshape(128, w_j * 128))
            off += w_j
    sel_np = sel_np.astype(bf16)

    # hd prefix actually referenced by the gather chunks (idx is monotone)
    hd_hi = int(idx_full[:, :512 * SEL_START].max()) + 1
    hd_hi = min(((hd_hi + 511) // 512) * 512, kmax)

    nc = _get_program(kmax, sel_key, hd_hi)
    in_maps = []
    for c in range(NCORES):
        b, h = divmod(c, 2)
        in_maps.append({
            "x": np.ascontiguousarray(
                hidden_states[b, :kmax, h * DSH:(h + 1) * DSH]).astype(bf16),
            "arow": arow[b],
            "pbcm": pbcm[b],
            "idx16": idx16[b],
            "sel": sel_np[b],
            "ident": np.eye(128, dtype=np.float32),
        })
    if trace:
        _install_profile_hook()
    res = run_bass_kernel_spmd(nc, in_maps, list(range(NCORES)), trace=trace)
    outs = res.results
    full = np.empty((B, L, D), np.float32)
    for c in range(NCORES):
        b, h = divmod(c, 2)
        full[b, :, h * DSH:(h + 1) * DSH] = outs[c]["out"].astype(np.float32)
    return full, res.exec_time_ns


def kernel(**inputs) -> np.ndarray:
    out, _ = run(inputs, trace=False)
    return out
